# revision 50
# baseline (speedup 1.0000x reference)
"""EGConv layer (gnn_message_passing) on 8 Trainium2 NeuronCores.

Self-contained: kernel(**inputs) -> np.ndarray [50000, 256] float32.

Strategy: graph-aligned 1D node partition over 8 cores (GraphNorm fully
core-local), per-core degree-sorted node permutation, dst-sorted edge
streams. Each core computes the bases rows of only ITS nodes (plus comb
weights) from its local node shard; a device AllGather assembles the
full [8Q, BF] bf16 bases table on every core. Messages are fetched by
dma_gather from two int16-indexable halves of that table (split at the
core-aligned row 4Q); sum+sym aggregation fused into one matmul per
edge tile (moving operand = [raw | symw-weighted] message planes); max
via slot-layout gather + halve + strided max-reduce; per-node einsum in
bf16 on the vector engine; GraphNorm stats via per-graph one-hot
matmuls, normalization constants fetched per node by a stats-table
dma_gather. Host<->device traffic is minimized: all inputs packed into
ONE bf16-typed blob (i16/f32 sections bitcast on device), gather-index
streams shipped 16-row (replicated to 128 on device), constant rows
shipped once and partition-broadcast, output returned as int8 with
per-node scales. The SPMD program is identical across cores; all
per-core variation is in the data.
"""
import sys
for _p in ("/opt/trn_rl_repo", "/root/.axon_site/_ro/trn_rl_repo"):
    if _p not in sys.path:
        sys.path.insert(0, _p)

import os
import numpy as np
import ml_dtypes
from contextlib import ExitStack

import concourse.bass as bass
import concourse.mybir as mybir
import concourse.tile as tile
from concourse import bacc, bass_utils

BFNP = ml_dtypes.bfloat16

# ======================= host-side graph preprocessing =======================

N, E, D = 50000, 800000, 256
H, B, A = 8, 4, 3
F = D // H          # 32
BF = B * F          # 128
G = 64
EPS = 1e-5
NCORES = 8
P = 128
NEG = -1e30


def build(edge_index: np.ndarray, batch: np.ndarray):
    """edge_index [2,E] int32, batch [N] int32 sorted. Returns layout dict."""
    src_all = np.concatenate([edge_index[0], np.arange(N, dtype=np.int64)])
    dst_all = np.concatenate([edge_index[1], np.arange(N, dtype=np.int64)])

    deg = np.bincount(dst_all, minlength=N).astype(np.float64)
    dinv = np.where(deg > 0, 1.0 / np.sqrt(deg), 0.0).astype(np.float32)
    symw_all = (dinv[src_all] * dinv[dst_all]).astype(np.float32)

    # graph-aligned 8-way shard
    gcnt = np.bincount(batch, minlength=G)
    gend = np.cumsum(gcnt)            # node index where graph g ends
    cuts = [0]
    for c in range(1, NCORES):
        target = round(N * c / NCORES)
        gi = np.argmin(np.abs(gend - target))
        cuts.append(int(gend[gi]))
    cuts.append(N)
    cuts = sorted(set(cuts))
    assert len(cuts) == NCORES + 1, cuts
    cuts_a = np.asarray(cuts, dtype=np.int64)
    src_core = np.searchsorted(cuts_a, src_all, side="right") - 1

    # pass 1: per-core degree-sorted permutation
    cores = []
    for c in range(NCORES):
        n0, n1 = cuts[c], cuts[c + 1]
        nloc = n1 - n0
        local_deg = deg[n0:n1]
        # secondary key: T0-range in-degree (srcs on cores 0-3), to tighten
        # per-range slot rectangles
        ldeg0 = np.bincount(dst_all[(dst_all >= n0) & (dst_all < n1)
                                    & (src_core < 4)] - n0,
                            minlength=nloc).astype(np.float64)
        perm = np.lexsort((-ldeg0, -local_deg)).astype(np.int64)  # desc
        gperm = perm + n0                      # new local id -> global id
        inv = np.empty(nloc, dtype=np.int64)
        inv[perm] = np.arange(nloc)            # orig local -> new local id
        cores.append(dict(n0=n0, n1=n1, nloc=nloc, gperm=gperm, inv=inv))

    maxloc = max(c["nloc"] for c in cores)
    Q = (maxloc // P + 1) * P                  # strictly > every nloc
    nblk = Q // P
    SPL = 4 * Q                                # T0/T1 split row (core-aligned)
    assert SPL - 1 <= 32767 and 4 * Q - 1 <= 32767

    ginv = np.empty(N, dtype=np.int64)         # orig global -> permuted row
    for c, core in enumerate(cores):
        ginv[core["gperm"]] = c * Q + np.arange(core["nloc"])

    # pass 2: per-core edge streams + global Tr/Sr
    nR = 2
    for core in cores:
        n0, n1 = core["n0"], core["n1"]
        emask = (dst_all >= n0) & (dst_all < n1)
        esrc = src_all[emask]
        edstl = core["inv"][dst_all[emask] - n0]   # new local dst id
        esym = symw_all[emask]
        order = np.argsort(edstl, kind="stable")
        core["esrc"], core["edstl"], core["esym"] = \
            esrc[order], edstl[order], esym[order]
        core["erow"] = ginv[core["esrc"]]          # permuted source row

    Tr = np.zeros((nR, nblk), dtype=np.int64)
    Sr = np.zeros((nR, nblk), dtype=np.int64)
    for c in cores:
        blk = c["edstl"] // P
        rng = (c["erow"] >= SPL).astype(np.int64)
        for r in range(nR):
            cnt = np.bincount(blk[rng == r], minlength=nblk)
            Tr[r] = np.maximum(Tr[r], (cnt + P - 1) // P)
            dl = c["edstl"][rng == r]
            nd = np.bincount(dl, minlength=nblk * P).reshape(nblk, P)
            Sr[r] = np.maximum(Sr[r], nd.max(axis=1))
    Tr = np.maximum(Tr, 1)
    Sr = np.maximum(Sr, 1)

    PAD0, PAD1 = SPL - 1, 4 * Q - 1   # NEG tail rows (cores 3 / 7), per-range

    sumTT = int((Tr[0] + Tr[1]).sum())
    for c in cores:
        nloc = c["nloc"]
        dstl_t = np.full((P, sumTT), -1.0, dtype=BFNP)
        symw_t = np.zeros((P, sumTT), dtype=BFNP)
        flat_r = [[]]          # single per-block-interleaved stream
        blk = c["edstl"] // P
        rng = (c["erow"] >= SPL).astype(np.int64)
        tcol = 0
        for b in range(nblk):
            for r in range(nR):
                m = (blk == b) & (rng == r)
                srow = c["erow"][m] - (SPL if r else 0)
                dl = c["edstl"][m] - b * P
                sw = c["esym"][m]
                k = len(srow)
                T, S = int(Tr[r][b]), int(Sr[r][b])
                pad = PAD1 if r else PAD0
                ef = np.full(P * T, pad, dtype=np.int64)
                ef[:k] = srow
                flat_r[0].append(ef)
                cols = tcol + np.arange(k) // P
                rows = np.arange(k) % P
                dstl_t[rows, cols] = dl.astype(np.float32)
                symw_t[rows, cols] = sw
                tcol += T
                sf = np.full(P * S, pad, dtype=np.int64)
                if k:
                    marks = np.flatnonzero(np.diff(dl, prepend=-1))
                    slot = np.arange(k) - np.repeat(marks, np.diff(
                        np.append(marks, k)))
                    sf[slot * P + dl] = srow
                    # pad slots of nodes that HAVE >=1 edge in this range:
                    # duplicate the node's first edge (max unchanged, avoids
                    # a NEG-row fetch)
                    first = np.full(P, -1, dtype=np.int64)
                    first[dl[marks]] = srow[marks]
                    degr = np.zeros(P, dtype=np.int64)
                    dcnt = np.diff(np.append(marks, k))
                    degr[dl[marks]] = dcnt
                    for s in range(S):
                        lane = np.flatnonzero((degr > 0) & (degr <= s))
                        sf[s * P + lane] = first[lane]
                flat_r[0].append(sf)
        fl = np.concatenate(flat_r[0])
        L = len(fl)
        i16_01 = np.zeros((16, L // 16), dtype=np.int16)
        i16_01[np.arange(L) % 16, np.arange(L) // 16] = fl

        gl0 = batch[c["n0"]]
        ngid = np.full(nblk * P, -1.0, dtype=np.float32)
        ngid[:nloc] = (batch[c["gperm"]] - gl0).astype(np.float32)
        gid_t = ngid.reshape(nblk, P).T.copy()

        # stage-E stats-gather index stream (graph id per node, block-major)
        gfl = np.where(ngid < 0, 0, ngid).astype(np.int64)
        L2f = nblk * P
        w2 = np.zeros((16, L2f // 16), dtype=np.int16)
        w2[np.arange(L2f) % 16, np.arange(L2f) // 16] = gfl
        gidx16 = w2

        nmv = np.zeros(nblk * P, dtype=np.float32)
        nmv[nloc:] = NEG
        nmask_t = nmv.reshape(nblk, P).T.copy()

        icnt = np.ones((G, 1), dtype=np.float32)
        glo = np.bincount((batch[c["n0"]:c["n1"]] - gl0), minlength=G)
        icnt[glo > 0, 0] = (1.0 / glo[glo > 0]).astype(np.float32)
        invc = np.ones((G, 1), dtype=np.float32)
        invc[:icnt.shape[0]] = icnt

        c.update(dstl_t=dstl_t, symw_t=symw_t, i16_01=i16_01,
                 gidx16=gidx16, gid_t=gid_t, nmask_t=nmask_t, invcnt=invc)

    return dict(cores=cores, nblk=nblk, Q=Q, Tr=Tr, Sr=Sr, cuts=cuts)


def unshard(layout, per_core_out):
    full = np.zeros((N, D), dtype=np.float32)
    for c, (q, s) in zip(layout["cores"], per_core_out):
        nloc = c["nloc"]
        sv = np.asarray(s, np.float32).T.reshape(-1)[:nloc]   # [P,nblk]->node
        full[c["gperm"]] = (np.asarray(q[:nloc], np.float32)
                            * (sv[:, None] / 127.0))
    return full

# ============ input-map construction ============


def to_bf16(x):
    return np.asarray(x, np.float32).astype(BFNP)


def make_inputs(inputs, lay):
    """inputs: dict of full np arrays. lay: build output.
    Returns (meta, in_maps list of 8 dicts)."""
    Q = lay["Q"]
    nblk = lay["nblk"]

    node = np.asarray(inputs["node"], np.float32)
    wb = to_bf16(inputs["W_bases"])                       # [D, BF]
    wc = to_bf16(inputs["W_comb"])                        # [D, HBA]
    bcomb = np.asarray(inputs["b_comb"], np.float32)      # [HBA]
    cbias = np.asarray(inputs["conv_bias"], np.float32)   # [D]
    alphar = np.asarray(inputs["gn_mean_scale"], np.float32)
    gammar = np.asarray(inputs["gn_weight"], np.float32)
    br = np.asarray(inputs["gn_bias"], np.float32)

    meta = dict(Q=Q, nblk=nblk,
                Tr0=[int(x) for x in lay["Tr"][0]],
                Tr1=[int(x) for x in lay["Tr"][1]],
                Sr0=[int(x) for x in lay["Sr"][0]],
                Sr1=[int(x) for x in lay["Sr"][1]])

    in_maps = []
    for c in lay["cores"]:
        ntl = np.zeros((D, Q), BFNP)
        ntl[:, :c["nloc"]] = to_bf16(node[c["gperm"]]).T
        blob_i16 = np.hstack([c["i16_01"], c["gidx16"]]).ravel()
        blob_f32 = np.concatenate([
            bcomb.ravel(), c["invcnt"].ravel(),
            alphar.ravel(), gammar.ravel(), br.ravel(),
            cbias.ravel()]).astype(np.float32)
        assert blob_f32.size % 2 == 0
        # int8 section: dstl (-1..127), gid (-1..63), pad flag (0/1)
        blob_i8 = np.concatenate([
            np.asarray(c["dstl_t"], np.float32).ravel(),
            c["gid_t"].ravel(),
            (c["nmask_t"].ravel() != 0.0).astype(np.float32)]
        ).astype(np.int8)
        assert blob_i8.size % 2 == 0
        blob = np.concatenate([
            ntl.ravel(), wb.ravel(), wc.ravel(), c["symw_t"].ravel(),
            blob_i16.view(BFNP), blob_f32.view(BFNP),
            blob_i8.view(BFNP)])
        in_maps.append(dict(blob=blob))
    return meta, in_maps

# ============ device program ============

FP32 = mybir.dt.float32
BF16 = mybir.dt.bfloat16
I32 = mybir.dt.int32
I16 = mybir.dt.int16
AX = mybir.AxisListType
OP = mybir.AluOpType
ACTF = mybir.ActivationFunctionType
HBA = H * B * A   # 96
K = B * A         # 12


def build_program(nc, meta):
    Q = meta["Q"]
    nblk = meta["nblk"]
    Tr0, Tr1 = list(meta["Tr0"]), list(meta["Tr1"])
    Sr0, Sr1 = list(meta["Sr0"]), list(meta["Sr1"])
    sumT = sum(Tr0) + sum(Tr1)
    L0 = sum(8 * (t + s) for t, s in zip(Tr0, Sr0))   # cols of [16, L0]
    L1 = sum(8 * (t + s) for t, s in zip(Tr1, Sr1))
    L2 = 8 * nblk                                      # stage-E gid stream
    TTmax = max(t0 + t1 for t0, t1 in zip(Tr0, Tr1))
    SPL = 4 * Q

    # ---- external blob (bf16-typed; i16/f32/i8 sections bitcast on device) -
    LBF = D * Q + D * BF + D * HBA + P * sumT
    LI = 16 * (L0 + L1 + L2)
    LF32 = HBA + G + 4 * D
    LI8 = P * sumT + 2 * P * nblk
    blob_bf = nc.dram_tensor("blob", [LBF + LI + 2 * LF32 + LI8 // 2], BF16,
                             kind="ExternalInput")
    # int8 output with per-node scales (values are >=0 post-relu)
    hout = nc.dram_tensor("hout", [Q, D], mybir.dt.int8, kind="ExternalOutput")
    houts = nc.dram_tensor("houts", [P, nblk], FP32, kind="ExternalOutput")

    o_ntl = 0
    o_wb = o_ntl + D * Q
    o_wc = o_wb + D * BF
    o_symw = o_wc + D * HBA
    i16_ap = blob_bf.ap()[LBF:LBF + LI].bitcast(I16)
    f32_ap = blob_bf.ap()[LBF + LI:LBF + LI + 2 * LF32].bitcast(FP32)
    i8_ap = blob_bf.ap()[LBF + LI + 2 * LF32:
                         LBF + LI + 2 * LF32 + LI8 // 2].bitcast(mybir.dt.int8)
    f_bcomb = 0
    f_invc = f_bcomb + HBA
    f_alphar = f_invc + G
    f_gammar = f_alphar + D
    f_br = f_gammar + D
    f_cbias = f_br + D
    i_dstl = 0
    i_gid = i_dstl + P * sumT
    i_nm = i_gid + P * nblk

    with ExitStack() as ctx:
        tc = ctx.enter_context(tile.TileContext(nc))
        dram = ctx.enter_context(tc.tile_pool(name="dram", bufs=1, space="DRAM"))
        res = ctx.enter_context(tc.tile_pool(name="res", bufs=1))
        pa = ctx.enter_context(tc.tile_pool(name="pa", bufs=3))
        pgath = ctx.enter_context(tc.tile_pool(name="pgath", bufs=2))
        pidx = ctx.enter_context(tc.tile_pool(name="pidx", bufs=2))
        poh = ctx.enter_context(tc.tile_pool(name="poh", bufs=2))
        ptmp = ctx.enter_context(tc.tile_pool(name="ptmp", bufs=2))
        psm = ctx.enter_context(tc.tile_pool(name="psm", bufs=2))
        pd = ctx.enter_context(tc.tile_pool(name="pd", bufs=1))

        # ---- constants / resident tiles ----------------------------------
        wb_sb = res.tile([P, 2, BF], BF16)
        nc.sync.dma_start(wb_sb[:], blob_bf.ap()[o_wb:o_wb + D * BF]
                          .rearrange("(a p f) -> p a f", p=P, f=BF))
        wc_sb = res.tile([P, 2, HBA], BF16)
        nc.sync.dma_start(wc_sb[:], blob_bf.ap()[o_wc:o_wc + D * HBA]
                          .rearrange("(a p f) -> p a f", p=P, f=HBA))
        dstl_i8 = res.tile([P, sumT], mybir.dt.int8)
        nc.sync.dma_start(dstl_i8[:], i8_ap[i_dstl:i_dstl + P * sumT]
                          .rearrange("(p t) -> p t", p=P))
        dstl_sb = res.tile([P, sumT], BF16)
        nc.vector.tensor_copy(dstl_sb[:], dstl_i8[:])
        symw_bf = res.tile([P, sumT], BF16)
        nc.sync.dma_start(symw_bf[:], blob_bf.ap()[o_symw:o_symw + P * sumT]
                          .rearrange("(p t) -> p t", p=P))
        symw_sb = res.tile([P, sumT], FP32)   # scalar-engine scale must be FP32
        nc.vector.tensor_copy(symw_sb[:], symw_bf[:])
        bcomb_sb = res.tile([P, HBA], FP32)
        nc.sync.dma_start(bcomb_sb[0:1, :], f32_ap[f_bcomb:f_bcomb + HBA]
                          .rearrange("(p t) -> p t", p=1))
        nc.gpsimd.partition_broadcast(bcomb_sb[:], bcomb_sb[0:1, :])
        gid_i8 = res.tile([P, nblk], mybir.dt.int8)
        nc.sync.dma_start(gid_i8[:], i8_ap[i_gid:i_gid + P * nblk]
                          .rearrange("(p t) -> p t", p=P))
        gid_sb = res.tile([P, nblk], FP32)
        nc.vector.tensor_copy(gid_sb[:], gid_i8[:])
        invc_sb = res.tile([G, 1], FP32)
        nc.sync.dma_start(invc_sb[:], f32_ap[f_invc:f_invc + G]
                          .rearrange("(p t) -> p t", p=G))
        alphar_sb = res.tile([G, D], FP32)
        nc.sync.dma_start(alphar_sb[0:1, :], f32_ap[f_alphar:f_alphar + D]
                          .rearrange("(p t) -> p t", p=1))
        nc.gpsimd.partition_broadcast(alphar_sb[:], alphar_sb[0:1, :])
        gammar_sb = res.tile([G, D], FP32)
        nc.sync.dma_start(gammar_sb[0:1, :], f32_ap[f_gammar:f_gammar + D]
                          .rearrange("(p t) -> p t", p=1))
        nc.gpsimd.partition_broadcast(gammar_sb[:], gammar_sb[0:1, :])
        br_sb = res.tile([G, D], FP32)
        nc.sync.dma_start(br_sb[0:1, :], f32_ap[f_br:f_br + D]
                          .rearrange("(p t) -> p t", p=1))
        nc.gpsimd.partition_broadcast(br_sb[:], br_sb[0:1, :])
        cbias_sb = res.tile([P, D], FP32)
        nc.sync.dma_start(cbias_sb[0:1, :], f32_ap[f_cbias:f_cbias + D]
                          .rearrange("(p t) -> p t", p=1))
        nc.gpsimd.partition_broadcast(cbias_sb[:], cbias_sb[0:1, :])
        nm_i8 = res.tile([P, nblk], mybir.dt.int8)
        nc.sync.dma_start(nm_i8[:], i8_ap[i_nm:i_nm + P * nblk]
                          .rearrange("(p t) -> p t", p=P))
        nmask_sb = res.tile([P, nblk], FP32)
        nc.vector.tensor_copy(nmask_sb[:], nm_i8[:])
        nc.vector.tensor_scalar(nmask_sb[:], nmask_sb[:], NEG, None,
                                op0=OP.mult)

        iota_i = res.tile([P, P], I32)
        nc.gpsimd.iota(iota_i[:], pattern=[[1, P]], base=0, channel_multiplier=0)
        iota_f = res.tile([P, P], FP32)
        nc.vector.tensor_copy(iota_f[:], iota_i[:])
        iota_bf = res.tile([P, P], BF16)
        nc.vector.tensor_copy(iota_bf[:], iota_i[:])
        iota_exp = res.tile([P, P, TTmax], BF16)
        nc.scalar.copy(iota_exp[:],
                       iota_bf[:].unsqueeze(2).broadcast_to([P, P, TTmax]))

        comb_sb = res.tile([P, nblk, HBA], BF16)
        hb_all = res.tile([P, nblk, D], BF16)
        hsc_all = res.tile([P, nblk], FP32)

        # ---- 16->128 replication of gather-index streams ------------------
        # layout: [0,L0) range0, [L0,L0+L1) range1, [L0+L1,..) stage-E gids
        rep = dram.tile([P, L0 + L1 + L2], I16)
        for kk in range(8):
            nc.sync.dma_start(rep[16 * kk:16 * (kk + 1), :],
                              i16_ap.rearrange("(a l) -> a l", a=16))

        # ---- stage A: local bases segment + comb, then AllGather ----------
        mybases = dram.tile([Q, BF], BF16)
        bases_all = nc.dram_tensor("bases_all", [NCORES * Q, BF], BF16,
                                   kind="Internal", addr_space="Shared")
        pab_cm = tc.tile_pool(name="pab", bufs=4, space="PSUM")
        pab = pab_cm.__enter__()
        pcb_cm = tc.tile_pool(name="pcb", bufs=2, space="PSUM")
        pcb = pcb_cm.__enter__()

        ntl_ap = blob_bf.ap()[o_ntl:o_ntl + D * Q].rearrange(
            "(a p n) -> p a n", p=P, n=Q)
        for b0 in range(0, nblk, 2):
            bn = min(2, nblk - b0)
            lt2 = pa.tile([P, 2, 2 * P], BF16, tag="lt")
            nc.sync.dma_start(lt2[:, :, :bn * P],
                              ntl_ap[:, :, b0 * P:(b0 + bn) * P])
            for j in range(bn):
                b = b0 + j
                ps = pab.tile([P, BF], FP32, tag="ab")
                nc.tensor.matmul(ps[:], lt2[:, 0, j * P:(j + 1) * P],
                                 wb_sb[:, 0, :], start=True, stop=False)
                nc.tensor.matmul(ps[:], lt2[:, 1, j * P:(j + 1) * P],
                                 wb_sb[:, 1, :], start=False, stop=True)
                ob = pa.tile([P, BF], BF16, tag="ob")
                nc.vector.tensor_scalar(ob[:], ps[:], nmask_sb[:, b:b + 1],
                                        None, op0=OP.add)
                nc.sync.dma_start(mybases[b * P:(b + 1) * P, :], ob[:])
                cps = pcb.tile([P, HBA], FP32, tag="cps")
                nc.tensor.matmul(cps[:], lt2[:, 0, j * P:(j + 1) * P],
                                 wc_sb[:, 0, :], start=True, stop=False)
                nc.tensor.matmul(cps[:], lt2[:, 1, j * P:(j + 1) * P],
                                 wc_sb[:, 1, :], start=False, stop=True)
                nc.vector.tensor_tensor(comb_sb[:, b, :], cps[:],
                                        bcomb_sb[:], op=OP.add)

        pcb_cm.__exit__(None, None, None)
        pab_cm.__exit__(None, None, None)

        nc.gpsimd.collective_compute(
            "AllGather", OP.bypass,
            replica_groups=[list(range(NCORES))],
            ins=[mybases[:].opt()],
            outs=[bases_all.ap().opt()])

        # ---- stage C: gather + aggregate + einsum + stats -----------------
        pacc_cm = tc.tile_pool(name="pacc", bufs=1, space="PSUM")
        pacc = pacc_cm.__enter__()
        pagg_cm = tc.tile_pool(name="pagg", bufs=2, space="PSUM")
        pagg = pagg_cm.__enter__()
        gsum_ps = pacc.tile([G, D], FP32)
        gsq_ps = pacc.tile([G, D], FP32)

        CH = 64                       # <=8192 idxs per dma_gather call
        c0 = 0
        tb = 0
        for b in range(nblk):
            T0, T1 = Tr0[b], Tr1[b]
            S0, S1 = Sr0[b], Sr1[b]
            W0, W1 = T0 + S0, T1 + S1
            TT = T0 + T1
            # gw[:, 1, :, :] = gathered messages; gw[:, 0, tile cols, :] =
            # symw-weighted messages (slot cols of plane 0 unused)
            gw = pgath.tile([P, 2, W0 + W1, BF], BF16, tag="gath")
            if b < 2:
                nc.gpsimd.memset(gw[:], 0.0)
            ix = pidx.tile([P, 8 * (W0 + W1)], I16, tag="ix")
            nc.sync.dma_start(ix[:], rep[:, c0:c0 + 8 * (W0 + W1)])
            for w0 in range(0, W0, CH):
                w = min(CH, W0 - w0)
                nc.gpsimd.dma_gather(
                    out_ap=gw[:, 1, w0:w0 + w, :], in_ap=bases_all.ap()[0:SPL, :],
                    idxs_ap=ix[:, 8 * w0:8 * (w0 + w)],
                    num_idxs=P * w, num_idxs_reg=P * w, elem_size=BF,
                    single_packet=False)
            for w1 in range(0, W1, CH):
                w = min(CH, W1 - w1)
                nc.gpsimd.dma_gather(
                    out_ap=gw[:, 1, W0 + w1:W0 + w1 + w, :],
                    in_ap=bases_all.ap()[SPL:NCORES * Q, :],
                    idxs_ap=ix[:, 8 * (W0 + w1):8 * (W0 + w1 + w)],
                    num_idxs=P * w, num_idxs_reg=P * w, elem_size=BF,
                    single_packet=False)
            c0 += 8 * (W0 + W1)

            # weighted copies of the tile columns (slot cols skipped)
            nc.vector.tensor_tensor(
                gw[:, 0, 0:T0, :], gw[:, 1, 0:T0, :],
                symw_sb[:, tb:tb + T0].unsqueeze(2).broadcast_to([P, T0, BF]),
                op=OP.mult)
            nc.vector.tensor_tensor(
                gw[:, 0, W0:W0 + T1, :], gw[:, 1, W0:W0 + T1, :],
                symw_sb[:, tb + T0:tb + TT].unsqueeze(2)
                .broadcast_to([P, T1, BF]),
                op=OP.mult)

            # block-level one-hot builds: oh[p_edge, x, t]
            oh = poh.tile([P, P, TTmax], BF16, tag="oh")
            nc.vector.tensor_tensor(
                oh[:, :, :TT],
                dstl_sb[:, tb:tb + TT].unsqueeze(1).broadcast_to([P, P, TT]),
                iota_exp[:, :, :TT], op=OP.is_equal)

            # one matmul per edge tile: moving [P, 2, BF] = (msg | w*msg)
            ps_c = pagg.tile([P, 2, BF], FP32, tag="aggc")
            for t in range(TT):
                mcol = t if t < T0 else W0 + (t - T0)
                nc.tensor.matmul(ps_c[:], oh[:, :, t], gw[:, :, mcol, :],
                                 start=(t == 0), stop=(t == TT - 1))
            # max: halve (overlap-safe) then one strided reduce over both
            # ranges into aggT[:, :, 8:12]
            m0, m1 = (S0 + 1) // 2, (S1 + 1) // 2
            hmax = ptmp.tile([P, m0 + m1, BF], BF16, tag="hmax")
            nc.vector.tensor_tensor(hmax[:, :m0, :],
                                    gw[:, 1, T0:T0 + m0, :],
                                    gw[:, 1, T0 + S0 - m0:T0 + S0, :],
                                    op=OP.max)
            nc.vector.tensor_tensor(hmax[:, m0:m0 + m1, :],
                                    gw[:, 1, W0 + T1:W0 + T1 + m1, :],
                                    gw[:, 1, W0 + W1 - m1:W0 + W1, :],
                                    op=OP.max)
            aggT = psm.tile([P, F, K], BF16, tag="aggT")
            nc.vector.tensor_reduce(
                aggT[:, :, 2 * B:3 * B].transpose([0, 2, 1]),
                hmax[:].rearrange("p s (bb f) -> p (bb f) s", bb=B),
                axis=AX.X, op=OP.max, opt_input=False)
            # sym (a=0) / sum (a=1) from psum, transposed to [P, F, b]
            nc.scalar.copy(
                aggT[:, :, 0:2 * B].rearrange("p f (a bb) -> p a bb f", a=2),
                ps_c[:].rearrange("p a (bb f) -> p a bb f", bb=B))

            # einsum: tmp[p,h,f,k] = aggT[p,f,k] * comb[p,h,k]; reduce k
            tmp = ptmp.tile([P, H, F, K], BF16, tag="tmp")
            nc.vector.tensor_tensor(
                tmp[:],
                aggT[:].unsqueeze(1).broadcast_to([P, H, F, K]),
                comb_sb[:, b, :].rearrange("p (h k) -> p h k", h=H)
                .unsqueeze(2).broadcast_to([P, H, F, K]),
                op=OP.mult)
            hbt = psm.tile([P, D], FP32, tag="hbt")
            nc.vector.tensor_reduce(hbt[:], tmp[:], axis=AX.X, op=OP.add,
                                    opt_input=False)
            nc.vector.tensor_tensor(hb_all[:, b, :], hbt[:], cbias_sb[:],
                                    op=OP.add)
            hsq = psm.tile([P, D], BF16, tag="hsq")
            nc.scalar.square(hsq[:], hb_all[:, b, :])

            # graph one-hot + stats
            goh = psm.tile([P, G], BF16, tag="goh")
            nc.vector.tensor_scalar(goh[:], iota_f[:, :G],
                                    gid_sb[:, b:b + 1], None, op0=OP.is_equal)
            nc.tensor.matmul(gsum_ps[:], goh[:], hb_all[:, b, :],
                             start=(b == 0), stop=(b == nblk - 1))
            nc.tensor.matmul(gsq_ps[:], goh[:], hsq[:],
                             start=(b == 0), stop=(b == nblk - 1))
            tb += TT

        # ---- stage D: per-graph stats ------------------------------------
        stats = res.tile([G, 2, D], BF16)    # q' | rstd*gamma
        mean = pd.tile([G, D], FP32, tag="mean")
        nc.vector.tensor_scalar(mean[:], gsum_ps[:], invc_sb[:, 0:1], None,
                                op0=OP.mult)
        ex2 = pd.tile([G, D], FP32, tag="ex2")
        nc.vector.tensor_scalar(ex2[:], gsq_ps[:], invc_sb[:, 0:1], None,
                                op0=OP.mult)
        meansc = pd.tile([G, D], FP32, tag="meansc")
        nc.vector.tensor_tensor(meansc[:], mean[:], alphar_sb[:], op=OP.mult)
        t2 = pd.tile([G, D], FP32, tag="t2")
        nc.vector.scalar_tensor_tensor(t2[:], mean[:], 2.0, meansc[:],
                                       op0=OP.mult, op1=OP.subtract)
        var = pd.tile([G, D], FP32, tag="var")
        nc.vector.tensor_tensor(var[:], meansc[:], t2[:], op=OP.mult)
        nc.vector.tensor_tensor(var[:], ex2[:], var[:], op=OP.subtract)
        nc.vector.tensor_scalar(var[:], var[:], EPS, None, op0=OP.add)
        sd = pd.tile([G, D], FP32, tag="sd")
        nc.scalar.activation(sd[:], var[:], ACTF.Sqrt)
        rstd = pd.tile([G, D], FP32, tag="rstd")
        nc.vector.reciprocal(rstd[:], sd[:])
        nc.vector.tensor_tensor(stats[:, 1, :], rstd[:], gammar_sb[:],
                                op=OP.mult)
        # q = meansc * (rstd*gamma) - beta  -> out = h*s - q
        nc.vector.tensor_tensor(stats[:, 0, :], meansc[:],
                                stats[:, 1, :], op=OP.mult)
        nc.vector.tensor_tensor(stats[:, 0, :], stats[:, 0, :],
                                br_sb[:], op=OP.subtract)

        # ---- stage E: normalize + relu + out -----------------------------
        # per-node (q, rstd*gamma) fetched by dma_gather from a G-row DRAM
        # stats table, indexed by the node's graph id
        pagg_cm.__exit__(None, None, None)
        pacc_cm.__exit__(None, None, None)
        dstats = dram.tile([G, 2 * D], BF16)
        nc.sync.dma_start(dstats[:], stats[:].rearrange("g a d -> g (a d)"))
        EB = 5
        for b0 in range(0, nblk, EB):
            eb = min(EB, nblk - b0)
            ixg = pidx.tile([P, 8 * EB], I16, tag="ixg")
            nc.sync.dma_start(ixg[:, :8 * eb],
                              rep[:, L0 + L1 + 8 * b0:L0 + L1 + 8 * (b0 + eb)])
            bcg = ptmp.tile([P, EB, 2 * D], BF16, tag="bcg")
            nc.gpsimd.dma_gather(
                out_ap=bcg[:, :eb, :], in_ap=dstats[:],
                idxs_ap=ixg[:, :8 * eb],
                num_idxs=P * eb, num_idxs_reg=P * eb, elem_size=2 * D,
                single_packet=False)
            for j in range(eb):
                b = b0 + j
                hc = psm.tile([P, D], FP32, tag="hc")
                nc.vector.tensor_tensor(hc[:], hb_all[:, b, :],
                                        bcg[:, j, D:2 * D], op=OP.mult)
                nc.vector.tensor_tensor(hc[:], hc[:], bcg[:, j, 0:D],
                                        op=OP.subtract)
                ho = psm.tile([P, D], FP32, tag="ho")
                nc.scalar.activation(ho[:], hc[:], ACTF.Relu)
                # per-node int8 quantization: s = rowmax, q = ho * 127/s
                nc.vector.tensor_reduce(hsc_all[:, b:b + 1], ho[:],
                                        axis=AX.X, op=OP.max, opt_input=False)
                nc.vector.tensor_scalar(hsc_all[:, b:b + 1],
                                        hsc_all[:, b:b + 1], 1e-12, None,
                                        op0=OP.max)
                rin = psm.tile([P, 1], FP32, tag="rin")
                nc.vector.reciprocal(rin[:], hsc_all[:, b:b + 1])
                qf = psm.tile([P, D], FP32, tag="qf")
                nc.vector.tensor_scalar(qf[:], ho[:], rin[:, 0:1], 127.0,
                                        op0=OP.mult, op1=OP.mult)
                qi = psm.tile([P, D], mybir.dt.int8, tag="qi")
                nc.vector.tensor_copy(qi[:], qf[:])
                nc.sync.dma_start(hout.ap()[b * P:(b + 1) * P, :], qi[:])
        nc.sync.dma_start(houts.ap(), hsc_all[:])

    return nc

# ======================= entry point =======================


def kernel(**inputs) -> np.ndarray:
    inputs = {k: np.asarray(v) for k, v in inputs.items()}
    lay = build(inputs["edge_index"].astype(np.int64),
                inputs["batch"].astype(np.int64))
    meta, in_maps = make_inputs(inputs, lay)

    nc = bacc.Bacc("TRN2", target_bir_lowering=False, debug=False,
                   num_devices=NCORES)
    build_program(nc, meta)
    nc.compile()
    res = bass_utils.run_bass_kernel_spmd(nc, in_maps,
                                          core_ids=list(range(NCORES)))
    outs = [(res.results[c]["hout"], res.results[c]["houts"])
            for c in range(NCORES)]
    kernel.last = dict(nc=nc, in_maps=in_maps, lay=lay, meta=meta)
    return unshard(lay, outs)
